# revision 25
# baseline (speedup 1.0000x reference)
# Trainium2 Bass kernel for nn_FMoELinearProj (moe_routing).
#
# Math: all fwd_expert_count values equal max_tokens (=4096), so the ragged
# scatter in the reference is a pure reshape and the whole op is, per expert k:
#     Out[:, k, :] = (X_k @ W_k^T + b_k) @ C_k
#                  = X_k @ (W_k^T C_k) + (b_k @ C_k)
# i.e. ONE [4096,256]x[256,64] GEMM per expert. The weight fold
# (W2_k = W_k^T C_k [256,64], bc_k = b_k C_k [64]) is ~0.5% of the FLOPs and
# runs on the host, as does the X transpose into the [d, token] layout the
# tensor engine wants and the bf16 casts (rel tolerance is 2e-2; bf16 keeps
# us ~5e-3).
#
# Device (per core, 8 experts): W2 is the stationary matmul operand
# ([128 d, 64 s] per d-chunk), X^T streams as [128 d, 512 tok] moving tiles.
# Two experts are column-tiled side-by-side in the PE array (tile_position
# (0,0)/(0,64)) so their N=512 matmuls run concurrently and fill one PSUM
# bank [128, 512]. DVE/ACT alternate psum eviction (folded-bias add + f32 ->
# bf16 downcast); output is written [j, s, t] bf16 and the host transposes/
# upcasts to the final [t, k, s] f32.
#
# The kernel is HBM-DMA bound (~21 MB/core at ~390-420 GB/s effective), so
# the structure optimizes the DMA stream: one 2 MB input DMA per expert on
# the sync HWDGE ring (FIFO per ring -> tiny w2/bc first), matmul batches
# issued in data-arrival order so compute fires as each tile lands, outputs
# on the separate scalar HWDGE ring, and the last expert's input/output
# split into 512/256 KB chunks so the tail pipelines with the stream end.
#
# Sharding: expert-parallel, 8 experts per NeuronCore, zero communication.

import numpy as np

K, TOK, D, E, S, P = 64, 4096, 256, 256, 64, 128
NCORE = 8
KL = K // NCORE          # experts per core
DC = D // P              # d-chunks (contraction split), = 2
TB = 512                 # tokens per matmul (moving-operand N)
NTB = TOK // TB          # token blocks per expert, = 8
NJP = KL // 2            # expert pairs per core, = 4

_CACHE = {}


def _build_nc():
    import concourse.tile as tile
    from concourse import bacc, mybir
    from contextlib import ExitStack

    f32 = mybir.dt.float32
    bf16 = mybir.dt.bfloat16

    nc = bacc.Bacc("TRN2", target_bir_lowering=False, debug=False,
                   num_devices=NCORE)
    xt_d = nc.dram_tensor("xt", [DC, P, KL, TOK], bf16, kind="ExternalInput").ap()
    w2_d = nc.dram_tensor("w2", [P, DC, KL, S], bf16, kind="ExternalInput").ap()
    bc_d = nc.dram_tensor("bc", [P, NJP], f32, kind="ExternalInput").ap()
    o_d = nc.dram_tensor("o", [KL, S, TOK], bf16, kind="ExternalOutput").ap()
    o_r = o_d.rearrange("(jj two) s t -> jj (two s) t", two=2)  # [NJP,128,TOK]
    xt_r = xt_d.rearrange("dc p j t -> j p dc t")               # [KL,128,DC,TOK]
    xt_q = xt_d.rearrange("dc p j (ck tq) -> j ck p dc tq", ck=4)

    with tile.TileContext(nc) as tc, ExitStack() as ctx:
        pc = ctx.enter_context(tc.tile_pool(name="consts", bufs=1))
        px = ctx.enter_context(tc.tile_pool(name="xin", bufs=4))
        pst = ctx.enter_context(tc.tile_pool(name="stg", bufs=4))
        pp = ctx.enter_context(tc.tile_pool(name="ps", bufs=8, space="PSUM"))

        copy_fn = mybir.ActivationFunctionType.Identity
        QT = TOK // 4            # output DMA quarter, for tail overlap

        # Tiny all-blocking weight/bias transfers MUST go first: HWDGE DMAs
        # complete in FIFO order per issuing engine.
        w2_sb = pc.tile([P, DC, KL, S], bf16)
        nc.sync.dma_start(out=w2_sb, in_=w2_d)
        bc_sb = pc.tile([P, NJP], f32)
        nc.sync.dma_start(out=bc_sb, in_=bc_d)

        def evict(st, ssl, po, jp):
            # DVE and ACT alternate psum evictions (bias add + downcast).
            if evict.flip:
                nc.vector.tensor_scalar_add(st[:, ssl], po,
                                            bc_sb[:, jp:jp + 1])
            else:
                nc.scalar.activation(st[:, ssl], po, copy_fn,
                                     bias=bc_sb[:, jp:jp + 1])
            evict.flip = not evict.flip
        evict.flip = True

        for jp in range(NJP):
            j0, j1 = 2 * jp, 2 * jp + 1
            last_pair = jp == NJP - 1
            # One 2 MB DMA per expert: both d-chunks land together.
            xj0 = px.tile([P, DC, TOK], bf16, tag="xj", name=f"xj0_{jp}", bufs=4)
            nc.sync.dma_start(out=xj0, in_=xt_r[j0])
            rhs0 = lambda dc, sl: xj0[:, dc, sl]
            if not last_pair:
                xj1 = px.tile([P, DC, TOK], bf16, tag="xj", name=f"xj1_{jp}", bufs=4)
                nc.sync.dma_start(out=xj1, in_=xt_r[j1])
                rhs1 = lambda dc, sl: xj1[:, dc, sl]
            else:
                # Final expert arrives in shrinking chunks (3x512KB, 2x256KB)
                # so the last matmul batches pipeline with the stream tail and
                # the post-stream serial chain is one token block long.
                CHUNKS = ((0, 2), (2, 2), (4, 2), (6, 1), (7, 1))
                xq = []
                for ci, (t0, nt) in enumerate(CHUNKS):
                    t = px.tile([P, DC, nt * TB], bf16, tag="xq",
                                name=f"xq{ci}", bufs=5)
                    nc.sync.dma_start(
                        out=t, in_=xt_r[j1][:, :, t0 * TB:(t0 + nt) * TB])
                    xq.append(t)
            pos = [pp.tile([P, TB], f32, tag="po", name=f"po{jp}_{tb}")
                   for tb in range(NTB)]
            # Matmuls issue in arrival order (HWDGE completes in FIFO order)
            # so each batch fires as soon as its data lands and the PE stays
            # busy through the whole load window.
            for dc in range(DC):
                for tb in range(NTB):
                    sl = slice(tb * TB, (tb + 1) * TB)
                    nc.tensor.matmul(pos[tb][0:S], lhsT=w2_sb[:, dc, j0, :],
                                     rhs=rhs0(dc, sl),
                                     start=dc == 0, stop=dc == 1)
            if not last_pair:
                st = pst.tile([P, TOK], bf16, tag="st", name=f"st{jp}", bufs=2)
                for dc in range(DC):
                    for tb in range(NTB):
                        sl = slice(tb * TB, (tb + 1) * TB)
                        nc.tensor.matmul(pos[tb][S:P],
                                         lhsT=w2_sb[:, dc, j1, :],
                                         rhs=rhs1(dc, sl),
                                         start=dc == 0, stop=dc == 1)
                        if dc == 1:
                            evict(st, sl, pos[tb], jp)
                nc.scalar.dma_start(out=o_r[jp], in_=st)
            else:
                stq = [pst.tile([P, nt * TB], bf16, tag="stq",
                                name=f"stq{ci}", bufs=5)
                       for ci, (t0, nt) in enumerate(CHUNKS)]
                for ci, (t0, nt) in enumerate(CHUNKS):
                    for h in range(nt):
                        tb = t0 + h
                        hs = slice(h * TB, (h + 1) * TB)
                        nc.tensor.matmul(pos[tb][S:P],
                                         lhsT=w2_sb[:, 0, j1, :],
                                         rhs=xq[ci][:, 0, hs],
                                         start=True, stop=False)
                        nc.tensor.matmul(pos[tb][S:P],
                                         lhsT=w2_sb[:, 1, j1, :],
                                         rhs=xq[ci][:, 1, hs],
                                         start=False, stop=True)
                        evict(stq[ci], hs, pos[tb], jp)
                    nc.scalar.dma_start(
                        out=o_r[jp][:, t0 * TB:(t0 + nt) * TB], in_=stq[ci])
    nc.compile()
    return nc


def _get_nc():
    if "nc" not in _CACHE:
        _CACHE["nc"] = _build_nc()
    return _CACHE["nc"]


def _prep_in_maps(x, w, b, c):
    """Host-side fold + shard: returns run_bass_kernel_spmd in_maps."""
    import ml_dtypes
    bf16 = ml_dtypes.bfloat16

    # W2[k, d, s] = sum_e w[k, e, d] c[k, e, s];  bc[k, s] = sum_e b[k, e] c[k, e, s]
    w2 = np.matmul(w.transpose(0, 2, 1), c)               # [K, D, S] f32
    bc = np.matmul(b[:, None, :], c)[:, 0, :]             # [K, S] f32

    in_maps = []
    for m in range(NCORE):
        js = slice(m * KL, (m + 1) * KL)
        # xt[dc, dl, j, t] = x[(m*KL+j)*TOK + t, dc*128 + dl]  (bf16)
        xm = x[m * KL * TOK:(m + 1) * KL * TOK].astype(bf16)
        xt = np.ascontiguousarray(
            xm.reshape(KL, TOK, DC, P).transpose(2, 3, 0, 1))
        # w2l[dl, dc, j, s] = W2[m*KL+j, dc*128+dl, s]  (bf16)
        w2l = np.ascontiguousarray(
            w2[js].reshape(KL, DC, P, S).transpose(2, 1, 0, 3).astype(bf16))
        # bc2[p, jp]: partitions 0-63 expert 2jp, 64-127 expert 2jp+1 (f32)
        bc2 = np.ascontiguousarray(
            bc[js].reshape(NJP, 2, S).transpose(1, 2, 0).reshape(P, NJP)
            .astype(np.float32))
        in_maps.append({"xt": xt, "w2": w2l, "bc": bc2})
    return in_maps


def _gather_out(results):
    """[KL, S, TOK] bf16 per core -> [TOK, K, S] f32 full output."""
    full = np.concatenate([r["o"] for r in results], axis=0)   # [K, S, TOK]
    return np.ascontiguousarray(full.transpose(2, 0, 1)).astype(np.float32)


def _numpy_fallback(x, counts, w, b, c, mt):
    k = counts.shape[0]
    offs = np.concatenate([[0], np.cumsum(counts)]).astype(np.int64)
    pad = np.zeros((k, mt, x.shape[1]), np.float32)
    for j in range(k):
        cnt = int(counts[j])
        pad[j, :cnt] = x[offs[j]:offs[j] + cnt]
    y = np.einsum("ktd,ked->kte", pad, w) + b[:, None, :]
    valid = (np.arange(mt)[None, :] < counts[:, None])[..., None]
    y = np.where(valid, y, 0.0).transpose(1, 0, 2)
    return np.einsum("nkd,kds->nks", y, c).astype(np.float32)


def kernel(inp, fwd_expert_count, weight, bias, c_psuedo_inv, max_tokens):
    x = np.ascontiguousarray(np.asarray(inp, dtype=np.float32))
    w = np.ascontiguousarray(np.asarray(weight, dtype=np.float32))
    b = np.ascontiguousarray(np.asarray(bias, dtype=np.float32))
    c = np.ascontiguousarray(np.asarray(c_psuedo_inv, dtype=np.float32))
    counts = np.asarray(fwd_expert_count)
    mt = int(max_tokens)

    shapes_ok = (w.shape == (K, E, D) and c.shape == (K, E, S)
                 and b.shape == (K, E) and x.shape == (K * TOK, D)
                 and mt == TOK and bool((counts == mt).all()))
    if not shapes_ok:
        return _numpy_fallback(x, counts, w, b, c, mt)

    from concourse.bass_utils import run_bass_kernel_spmd
    nc = _get_nc()
    in_maps = _prep_in_maps(x, w, b, c)
    res = run_bass_kernel_spmd(nc, in_maps, core_ids=list(range(NCORE)))
    return _gather_out(res.results)


# revision 28
# speedup vs baseline: 1.0138x; 1.0138x over previous
# Trainium2 Bass kernel for nn_FMoELinearProj (moe_routing).
#
# Math: all fwd_expert_count values equal max_tokens (=4096), so the ragged
# scatter in the reference is a pure reshape and the whole op is, per expert k:
#     Out[:, k, :] = (X_k @ W_k^T + b_k) @ C_k
#                  = X_k @ (W_k^T C_k) + (b_k @ C_k)
# i.e. ONE [4096,256]x[256,64] GEMM per expert. The weight fold
# (W2_k = W_k^T C_k [256,64], bc_k = b_k C_k [64]) is ~0.5% of the FLOPs and
# runs on the host, as does the X transpose into the [d, token] layout the
# tensor engine wants and the bf16 casts (rel tolerance is 2e-2; bf16 keeps
# us ~5e-3).
#
# Device (per core, 8 experts): W2 is the stationary matmul operand
# ([128 d, 64 s] per d-chunk), X^T streams as [128 d, 512 tok] moving tiles.
# Two experts are column-tiled side-by-side in the PE array (tile_position
# (0,0)/(0,64)) so their N=512 matmuls run concurrently and fill one PSUM
# bank [128, 512]. DVE/ACT alternate psum eviction (folded-bias add + f32 ->
# bf16 downcast); output is written [j, s, t] bf16 and the host transposes/
# upcasts to the final [t, k, s] f32.
#
# The kernel is HBM-DMA bound (~21 MB/core at ~390-420 GB/s effective), so
# the structure optimizes the DMA stream: one 2 MB input DMA per expert on
# the sync HWDGE ring (FIFO per ring -> tiny w2/bc first), matmul batches
# issued in data-arrival order so compute fires as each tile lands, outputs
# on the separate scalar HWDGE ring, and the last expert's input/output
# split into 512/256 KB chunks so the tail pipelines with the stream end.
#
# Sharding: expert-parallel, 8 experts per NeuronCore, zero communication.

import numpy as np

K, TOK, D, E, S, P = 64, 4096, 256, 256, 64, 128
NCORE = 8
KL = K // NCORE          # experts per core
DC = D // P              # d-chunks (contraction split), = 2
TB = 512                 # tokens per matmul (moving-operand N)
NTB = TOK // TB          # token blocks per expert, = 8
NJP = KL // 2            # expert pairs per core, = 4

_CACHE = {}


def _build_nc():
    import concourse.tile as tile
    from concourse import bacc, mybir
    from contextlib import ExitStack

    f32 = mybir.dt.float32
    bf16 = mybir.dt.bfloat16

    nc = bacc.Bacc("TRN2", target_bir_lowering=False, debug=False,
                   num_devices=NCORE)
    xt_d = nc.dram_tensor("xt", [DC, P, KL, TOK], bf16, kind="ExternalInput").ap()
    w2_d = nc.dram_tensor("w2", [P, DC, KL, S], bf16, kind="ExternalInput").ap()
    bc_d = nc.dram_tensor("bc", [P, NJP], f32, kind="ExternalInput").ap()
    o_d = nc.dram_tensor("o", [KL, S, TOK], bf16, kind="ExternalOutput").ap()
    o_r = o_d.rearrange("(jj two) s t -> jj (two s) t", two=2)  # [NJP,128,TOK]
    xt_r = xt_d.rearrange("dc p j t -> j p dc t")               # [KL,128,DC,TOK]
    xt_q = xt_d.rearrange("dc p j (ck tq) -> j ck p dc tq", ck=4)

    with tile.TileContext(nc) as tc, ExitStack() as ctx:
        pc = ctx.enter_context(tc.tile_pool(name="consts", bufs=1))
        px = ctx.enter_context(tc.tile_pool(name="xin", bufs=4))
        pst = ctx.enter_context(tc.tile_pool(name="stg", bufs=4))
        pp = ctx.enter_context(tc.tile_pool(name="ps", bufs=8, space="PSUM"))

        copy_fn = mybir.ActivationFunctionType.Identity
        QT = TOK // 4            # output DMA quarter, for tail overlap

        # Weight/bias preload rides the scalar HWDGE ring so the sync ring's
        # very first issue is the X stream itself (the two rings' head issues
        # overlap; HWDGE completes in FIFO order per ring).
        w2_sb = pc.tile([P, DC, KL, S], bf16)
        nc.scalar.dma_start(out=w2_sb, in_=w2_d)
        bc_sb = pc.tile([P, NJP], f32)
        nc.scalar.dma_start(out=bc_sb, in_=bc_d)

        def evict(st, ssl, po, jp):
            # DVE and ACT alternate psum evictions (bias add + downcast).
            if evict.flip:
                nc.vector.tensor_scalar_add(st[:, ssl], po,
                                            bc_sb[:, jp:jp + 1])
            else:
                nc.scalar.activation(st[:, ssl], po, copy_fn,
                                     bias=bc_sb[:, jp:jp + 1])
            evict.flip = not evict.flip
        evict.flip = True

        for jp in range(NJP):
            j0, j1 = 2 * jp, 2 * jp + 1
            last_pair = jp == NJP - 1
            # One 2 MB DMA per expert: both d-chunks land together.
            xj0 = px.tile([P, DC, TOK], bf16, tag="xj", name=f"xj0_{jp}", bufs=4)
            nc.sync.dma_start(out=xj0, in_=xt_r[j0])
            rhs0 = lambda dc, sl: xj0[:, dc, sl]
            if not last_pair:
                xj1 = px.tile([P, DC, TOK], bf16, tag="xj", name=f"xj1_{jp}", bufs=4)
                nc.sync.dma_start(out=xj1, in_=xt_r[j1])
                rhs1 = lambda dc, sl: xj1[:, dc, sl]
            else:
                # Final expert arrives in shrinking chunks (3x512KB, 2x256KB)
                # so the last matmul batches pipeline with the stream tail and
                # the post-stream serial chain is one token block long.
                CHUNKS = ((0, 2), (2, 2), (4, 2), (6, 1), (7, 1))
                xq = []
                for ci, (t0, nt) in enumerate(CHUNKS):
                    t = px.tile([P, DC, nt * TB], bf16, tag="xq",
                                name=f"xq{ci}", bufs=5)
                    nc.sync.dma_start(
                        out=t, in_=xt_r[j1][:, :, t0 * TB:(t0 + nt) * TB])
                    xq.append(t)
            pos = [pp.tile([P, TB], f32, tag="po", name=f"po{jp}_{tb}")
                   for tb in range(NTB)]
            # Matmuls issue in arrival order (HWDGE completes in FIFO order)
            # so each batch fires as soon as its data lands and the PE stays
            # busy through the whole load window.
            for dc in range(DC):
                for tb in range(NTB):
                    sl = slice(tb * TB, (tb + 1) * TB)
                    nc.tensor.matmul(pos[tb][0:S], lhsT=w2_sb[:, dc, j0, :],
                                     rhs=rhs0(dc, sl),
                                     start=dc == 0, stop=dc == 1)
            if not last_pair:
                st = pst.tile([P, TOK], bf16, tag="st", name=f"st{jp}", bufs=2)
                for dc in range(DC):
                    for tb in range(NTB):
                        sl = slice(tb * TB, (tb + 1) * TB)
                        nc.tensor.matmul(pos[tb][S:P],
                                         lhsT=w2_sb[:, dc, j1, :],
                                         rhs=rhs1(dc, sl),
                                         start=dc == 0, stop=dc == 1)
                        if dc == 1:
                            evict(st, sl, pos[tb], jp)
                nc.scalar.dma_start(out=o_r[jp], in_=st)
            else:
                stq = [pst.tile([P, nt * TB], bf16, tag="stq",
                                name=f"stq{ci}", bufs=5)
                       for ci, (t0, nt) in enumerate(CHUNKS)]
                for ci, (t0, nt) in enumerate(CHUNKS):
                    for h in range(nt):
                        tb = t0 + h
                        hs = slice(h * TB, (h + 1) * TB)
                        nc.tensor.matmul(pos[tb][S:P],
                                         lhsT=w2_sb[:, 0, j1, :],
                                         rhs=xq[ci][:, 0, hs],
                                         start=True, stop=False)
                        nc.tensor.matmul(pos[tb][S:P],
                                         lhsT=w2_sb[:, 1, j1, :],
                                         rhs=xq[ci][:, 1, hs],
                                         start=False, stop=True)
                        evict(stq[ci], hs, pos[tb], jp)
                    nc.scalar.dma_start(
                        out=o_r[jp][:, t0 * TB:(t0 + nt) * TB], in_=stq[ci])
    nc.compile()
    return nc


def _get_nc():
    if "nc" not in _CACHE:
        _CACHE["nc"] = _build_nc()
    return _CACHE["nc"]


def _prep_in_maps(x, w, b, c):
    """Host-side fold + shard: returns run_bass_kernel_spmd in_maps."""
    import ml_dtypes
    bf16 = ml_dtypes.bfloat16

    # W2[k, d, s] = sum_e w[k, e, d] c[k, e, s];  bc[k, s] = sum_e b[k, e] c[k, e, s]
    w2 = np.matmul(w.transpose(0, 2, 1), c)               # [K, D, S] f32
    bc = np.matmul(b[:, None, :], c)[:, 0, :]             # [K, S] f32

    in_maps = []
    for m in range(NCORE):
        js = slice(m * KL, (m + 1) * KL)
        # xt[dc, dl, j, t] = x[(m*KL+j)*TOK + t, dc*128 + dl]  (bf16)
        xm = x[m * KL * TOK:(m + 1) * KL * TOK].astype(bf16)
        xt = np.ascontiguousarray(
            xm.reshape(KL, TOK, DC, P).transpose(2, 3, 0, 1))
        # w2l[dl, dc, j, s] = W2[m*KL+j, dc*128+dl, s]  (bf16)
        w2l = np.ascontiguousarray(
            w2[js].reshape(KL, DC, P, S).transpose(2, 1, 0, 3).astype(bf16))
        # bc2[p, jp]: partitions 0-63 expert 2jp, 64-127 expert 2jp+1 (f32)
        bc2 = np.ascontiguousarray(
            bc[js].reshape(NJP, 2, S).transpose(1, 2, 0).reshape(P, NJP)
            .astype(np.float32))
        in_maps.append({"xt": xt, "w2": w2l, "bc": bc2})
    return in_maps


def _gather_out(results):
    """[KL, S, TOK] bf16 per core -> [TOK, K, S] f32 full output."""
    full = np.concatenate([r["o"] for r in results], axis=0)   # [K, S, TOK]
    return np.ascontiguousarray(full.transpose(2, 0, 1)).astype(np.float32)


def _numpy_fallback(x, counts, w, b, c, mt):
    k = counts.shape[0]
    offs = np.concatenate([[0], np.cumsum(counts)]).astype(np.int64)
    pad = np.zeros((k, mt, x.shape[1]), np.float32)
    for j in range(k):
        cnt = int(counts[j])
        pad[j, :cnt] = x[offs[j]:offs[j] + cnt]
    y = np.einsum("ktd,ked->kte", pad, w) + b[:, None, :]
    valid = (np.arange(mt)[None, :] < counts[:, None])[..., None]
    y = np.where(valid, y, 0.0).transpose(1, 0, 2)
    return np.einsum("nkd,kds->nks", y, c).astype(np.float32)


def kernel(inp, fwd_expert_count, weight, bias, c_psuedo_inv, max_tokens):
    x = np.ascontiguousarray(np.asarray(inp, dtype=np.float32))
    w = np.ascontiguousarray(np.asarray(weight, dtype=np.float32))
    b = np.ascontiguousarray(np.asarray(bias, dtype=np.float32))
    c = np.ascontiguousarray(np.asarray(c_psuedo_inv, dtype=np.float32))
    counts = np.asarray(fwd_expert_count)
    mt = int(max_tokens)

    shapes_ok = (w.shape == (K, E, D) and c.shape == (K, E, S)
                 and b.shape == (K, E) and x.shape == (K * TOK, D)
                 and mt == TOK and bool((counts == mt).all()))
    if not shapes_ok:
        return _numpy_fallback(x, counts, w, b, c, mt)

    from concourse.bass_utils import run_bass_kernel_spmd
    nc = _get_nc()
    in_maps = _prep_in_maps(x, w, b, c)
    res = run_bass_kernel_spmd(nc, in_maps, core_ids=list(range(NCORE)))
    return _gather_out(res.results)
